# revision 5
# baseline (speedup 1.0000x reference)
"""ArcFace loss on Trainium2 — 8 NeuronCores, data-parallel over rows.

Math (per row, with S=30, M=0.5):
    cos_t  = clip(pred, -1, 1)
    t      = cos_t[target]
    tgt_m  = t*cos(M) - sin(M)*sqrt(1-t^2)   if t > cos(pi-M)
           = t - sin(M)*M                     otherwise
    logit  = S * cos_t, with target entry replaced by S*tgt_m
    loss   = logsumexp(logit) - S*tgt_m
    out    = mean(loss)

Since logit <= S always, logsumexp(logit) = S + log(sum_j exp(logit_j - S))
with a FIXED shift S — no per-row max needed.  The row sum streams through
ScalarE as exp(S*min(x,1) - S) with the free-axis accumulator (accum_out),
after one VectorE clip pass.  The target entry's term is subtracted and the
margin term added per-row at the end (tiny [128, G] tensors), using the
identity cos(arccos(t)+M) = t*cosM - sinM*sqrt(1-t^2), with
sqrt(u) = exp(0.5*ln(u)) to stay inside the natural_log_exp ACT table set.

Each core reduces its 1024 row losses to a single scalar (matmul with ones);
the host sums the 8 partial sums and divides by N.
"""

import math
import sys

import numpy as np

if "/opt/trn_rl_repo" not in sys.path:
    sys.path.insert(0, "/opt/trn_rl_repo")

S = 30.0
M = 0.5
COS_M = math.cos(M)
SIN_M = math.sin(M)
MM = math.sin(math.pi - M) * M
THRESHOLD = math.cos(math.pi - M)

N, C = 8192, 32000
N_CORES = 8
N_SHARD = N // N_CORES  # 1024 rows per core
P = 128  # SBUF partitions


def build_nc(n_shard=N_SHARD, n_classes=C, f_chunk=4000, in_bufs=3, dump_bufs=2):
    """Build the single-core Bass program (SPMD: same program on all cores)."""
    import concourse.bacc as bacc
    import concourse.tile as tile
    from concourse import bass, mybir

    f32 = mybir.dt.float32
    i32 = mybir.dt.int32
    Act = mybir.ActivationFunctionType
    Alu = mybir.AluOpType
    X = mybir.AxisListType.X

    G = n_shard // P        # row groups of 128
    K = n_classes // f_chunk  # column chunks
    assert G * P == n_shard and K * f_chunk == n_classes

    nc = bacc.Bacc(None, target_bir_lowering=False)
    pred = nc.declare_dram_parameter("pred", [n_shard, n_classes], f32, isOutput=False)
    target = nc.declare_dram_parameter("target", [n_shard], i32, isOutput=False)
    out = nc.declare_dram_parameter("out", [1, 1], f32, isOutput=True)

    with tile.TileContext(nc) as tc:
        with (
            tc.tile_pool(name="xin", bufs=in_bufs) as xin_pool,
            tc.tile_pool(name="edump", bufs=dump_bufs) as edump_pool,
            tc.tile_pool(name="persist", bufs=1) as persist,
            tc.tile_pool(name="psum", bufs=1, space="PSUM") as psum_pool,
        ):
            # --- persistent accumulators / epilogue tensors ---
            rs_partial = persist.tile([P, G * K], f32)  # per (group, chunk) row sums
            nc.vector.memset(rs_partial[:], 0.0)

            # activation bias must be a per-partition AP
            bias_neg_s = persist.tile([P, 1], f32)
            nc.vector.memset(bias_neg_s[:], -S)

            # target indices laid out [partition p, group g] = target[g*P + p]
            tgt_i = persist.tile([P, G], i32)
            nc.sync.dma_start(
                out=tgt_i[:], in_=target[:].rearrange("(g p) -> p g", p=P)
            )

            # flat gather indices within a group: p*C + target  (< 2^24, f32-exact)
            rowoff_i = persist.tile([P, 1], i32)
            nc.gpsimd.iota(rowoff_i[:], pattern=[[0, 1]], base=0,
                           channel_multiplier=n_classes)
            rowoff_f = persist.tile([P, 1], f32)
            nc.vector.tensor_copy(out=rowoff_f[:], in_=rowoff_i[:])
            tgt_f = persist.tile([P, G], f32)
            nc.vector.tensor_copy(out=tgt_f[:], in_=tgt_i[:])
            flat_f = persist.tile([P, G], f32)
            nc.vector.tensor_scalar(
                out=flat_f[:], in0=tgt_f[:], scalar1=rowoff_f[:, 0:1], scalar2=None,
                op0=Alu.add,
            )
            flat_i = persist.tile([P, G], i32)
            nc.vector.tensor_copy(out=flat_i[:], in_=flat_f[:])

            # gather pred[row, target[row]] for each row -> t_raw[p, g]
            pred_flat = pred[:].rearrange("n c -> (n c)")[:, None]
            t_raw = persist.tile([P, G], f32)
            for g in range(G):
                nc.gpsimd.indirect_dma_start(
                    out=t_raw[:, g:g + 1],
                    out_offset=None,
                    in_=pred_flat,
                    in_offset=bass.IndirectOffsetOnAxis(ap=flat_i[:, g:g + 1], axis=0),
                    element_offset=g * P * n_classes,
                )

            # --- hot loop: stream pred, clip-top, exp, accumulate row sums ---
            for g in range(G):
                for k in range(K):
                    x = xin_pool.tile([P, f_chunk], f32)
                    nc.sync.dma_start(
                        out=x[:],
                        in_=pred[g * P:(g + 1) * P, k * f_chunk:(k + 1) * f_chunk],
                    )
                    nc.vector.tensor_scalar_min(out=x[:], in0=x[:], scalar1=1.0)
                    e = edump_pool.tile([P, f_chunk], f32)
                    nc.scalar.activation(
                        out=e[:], in_=x[:], func=Act.Exp, bias=bias_neg_s[:], scale=S,
                        accum_out=rs_partial[:, g * K + k:g * K + k + 1],
                    )

            # --- epilogue (all on [P, G] / [P, 1] tensors) ---
            rs = persist.tile([P, G], f32)
            nc.vector.tensor_reduce(
                out=rs[:], in_=rs_partial[:].rearrange("p (g k) -> p g k", k=K),
                axis=X, op=Alu.add,
            )

            # e_t: the term the hot loop added for the target element
            m_t = persist.tile([P, G], f32)
            nc.vector.tensor_scalar_min(out=m_t[:], in0=t_raw[:], scalar1=1.0)
            e_t = persist.tile([P, G], f32)
            nc.scalar.activation(out=e_t[:], in_=m_t[:], func=Act.Exp,
                                 bias=bias_neg_s[:], scale=S)

            # tgt_cos = clip(t_raw, -1, 1)
            tgt_cos = persist.tile([P, G], f32)
            nc.vector.tensor_scalar(
                out=tgt_cos[:], in0=t_raw[:], scalar1=1.0, scalar2=-1.0,
                op0=Alu.min, op1=Alu.max,
            )
            # sqrt(1 - t^2) = exp(0.5*ln(max(1-t^2, eps)))
            u = persist.tile([P, G], f32)
            nc.vector.tensor_tensor(out=u[:], in0=tgt_cos[:], in1=tgt_cos[:],
                                    op=Alu.mult)
            nc.vector.tensor_scalar(
                out=u[:], in0=u[:], scalar1=-1.0, scalar2=1.0,
                op0=Alu.mult, op1=Alu.add,
            )  # u = 1 - t^2
            nc.vector.tensor_scalar_max(out=u[:], in0=u[:], scalar1=1e-12)
            lnu = persist.tile([P, G], f32)
            nc.scalar.activation(out=lnu[:], in_=u[:], func=Act.Ln)
            sq = persist.tile([P, G], f32)
            nc.scalar.activation(out=sq[:], in_=lnu[:], func=Act.Exp, scale=0.5)

            # tgt_m = where(tgt_cos > THRESHOLD, t*cosM - sinM*sq, tgt_cos - MM)
            cosm_t = persist.tile([P, G], f32)
            nc.vector.tensor_scalar_mul(out=cosm_t[:], in0=tgt_cos[:], scalar1=COS_M)
            tgt_m_raw = persist.tile([P, G], f32)
            nc.vector.scalar_tensor_tensor(
                out=tgt_m_raw[:], in0=sq[:], scalar=-SIN_M, op0=Alu.mult,
                in1=cosm_t[:], op1=Alu.add,
            )
            mask = persist.tile([P, G], mybir.dt.uint8)
            nc.vector.tensor_scalar(
                out=mask[:], in0=tgt_cos[:], scalar1=THRESHOLD, scalar2=None,
                op0=Alu.is_gt,
            )
            alt = persist.tile([P, G], f32)
            nc.vector.tensor_scalar_add(out=alt[:], in0=tgt_cos[:], scalar1=-MM)
            tgt_m = persist.tile([P, G], f32)
            nc.vector.select(out=tgt_m[:], mask=mask[:], on_true=tgt_m_raw[:],
                             on_false=alt[:])

            e_m = persist.tile([P, G], f32)
            nc.scalar.activation(out=e_m[:], in_=tgt_m[:], func=Act.Exp,
                                 bias=bias_neg_s[:], scale=S)

            # s' = rs - e_t + e_m ;  loss = S + ln(s') - S*tgt_m
            nc.vector.tensor_tensor(out=rs[:], in0=rs[:], in1=e_t[:],
                                    op=Alu.subtract)
            nc.vector.tensor_tensor(out=rs[:], in0=rs[:], in1=e_m[:], op=Alu.add)
            ln_s = persist.tile([P, G], f32)
            nc.scalar.activation(out=ln_s[:], in_=rs[:], func=Act.Ln)
            loss = persist.tile([P, G], f32)
            nc.vector.scalar_tensor_tensor(
                out=loss[:], in0=tgt_m[:], scalar=-S, op0=Alu.mult,
                in1=ln_s[:], op1=Alu.add,
            )
            nc.vector.tensor_scalar_add(out=loss[:], in0=loss[:], scalar1=S)

            # per-core scalar: sum over all rows = ones^T @ rowsums
            loss_rowsum = persist.tile([P, 1], f32)
            nc.vector.tensor_reduce(out=loss_rowsum[:], in_=loss[:], axis=X,
                                    op=Alu.add)
            ones = persist.tile([P, 1], f32)
            nc.vector.memset(ones[:], 1.0)
            ps = psum_pool.tile([1, 1], f32)
            nc.tensor.matmul(out=ps[:], lhsT=loss_rowsum[:], rhs=ones[:],
                             start=True, stop=True)
            out_s = persist.tile([1, 1], f32)
            nc.vector.tensor_copy(out=out_s[:], in_=ps[:])
            nc.sync.dma_start(out=out[:, :], in_=out_s[:])

    nc.finalize()
    return nc


_CACHE = {}


def _get_nc():
    if "nc" not in _CACHE:
        _CACHE["nc"] = build_nc()
    return _CACHE["nc"]


def kernel(pred, target):
    from concourse.bass_utils import run_bass_kernel_spmd

    pred = np.ascontiguousarray(np.asarray(pred, dtype=np.float32))
    tgt = np.ascontiguousarray(np.asarray(target).astype(np.int32))
    assert pred.shape == (N, C) and tgt.shape == (N,)

    in_maps = [
        {
            "pred": pred[c * N_SHARD:(c + 1) * N_SHARD],
            "target": tgt[c * N_SHARD:(c + 1) * N_SHARD],
        }
        for c in range(N_CORES)
    ]
    nc = _get_nc()
    res = run_bass_kernel_spmd(nc, in_maps, core_ids=list(range(N_CORES)))
    partials = [np.asarray(r["out"], dtype=np.float64).reshape(-1)[0]
                for r in res.results]
    return np.float32(np.sum(partials) / N)


# revision 6
# speedup vs baseline: 1.2802x; 1.2802x over previous
"""ArcFace loss on Trainium2 — 8 NeuronCores, data-parallel over rows.

Math (per row, with S=30, M=0.5):
    cos_t  = clip(pred, -1, 1)
    t      = cos_t[target]
    tgt_m  = t*cos(M) - sin(M)*sqrt(1-t^2)   if t > cos(pi-M)
           = t - sin(M)*M                     otherwise
    logit  = S * cos_t, with target entry replaced by S*tgt_m
    loss   = logsumexp(logit) - S*tgt_m
    out    = mean(loss)

Since logit <= S always, logsumexp(logit) = S + log(sum_j exp(logit_j - S))
with a FIXED shift S — no per-row max needed.  The row sum streams through
ScalarE as exp(S*min(x,1) - S) with the free-axis accumulator (accum_out),
after one VectorE clip pass.  The target entry's term is subtracted and the
margin term added per-row at the end (tiny [128, G] tensors), using the
identity cos(arccos(t)+M) = t*cosM - sinM*sqrt(1-t^2), with
sqrt(u) = exp(0.5*ln(u)) to stay inside the natural_log_exp ACT table set.

Each core reduces its 1024 row losses to a single scalar (matmul with ones);
the host sums the 8 partial sums and divides by N.
"""

import math
import sys

import numpy as np

if "/opt/trn_rl_repo" not in sys.path:
    sys.path.insert(0, "/opt/trn_rl_repo")

S = 30.0
M = 0.5
COS_M = math.cos(M)
SIN_M = math.sin(M)
MM = math.sin(math.pi - M) * M
THRESHOLD = math.cos(math.pi - M)

N, C = 8192, 32000
N_CORES = 8
N_SHARD = N // N_CORES  # 1024 rows per core
P = 128  # SBUF partitions


def build_nc(n_shard=N_SHARD, n_classes=C, f_chunk=4000, in_bufs=5, dump_bufs=3):
    """Build the single-core Bass program (SPMD: same program on all cores)."""
    import concourse.bacc as bacc
    import concourse.tile as tile
    from concourse import bass, mybir

    f32 = mybir.dt.float32
    i32 = mybir.dt.int32
    Act = mybir.ActivationFunctionType
    Alu = mybir.AluOpType
    X = mybir.AxisListType.X

    G = n_shard // P        # row groups of 128
    K = n_classes // f_chunk  # column chunks
    assert G * P == n_shard and K * f_chunk == n_classes

    nc = bacc.Bacc(None, target_bir_lowering=False)
    pred = nc.declare_dram_parameter("pred", [n_shard, n_classes], f32, isOutput=False)
    target = nc.declare_dram_parameter("target", [n_shard], i32, isOutput=False)
    out = nc.declare_dram_parameter("out", [1, 1], f32, isOutput=True)

    with tile.TileContext(nc) as tc:
        with (
            tc.tile_pool(name="xin", bufs=in_bufs) as xin_pool,
            tc.tile_pool(name="edump", bufs=dump_bufs) as edump_pool,
            tc.tile_pool(name="persist", bufs=1) as persist,
            tc.tile_pool(name="psum", bufs=1, space="PSUM") as psum_pool,
        ):
            # --- persistent accumulators / epilogue tensors ---
            rs_partial = persist.tile([P, G * K], f32)  # per (group, chunk) row sums
            nc.vector.memset(rs_partial[:], 0.0)

            # activation bias must be a per-partition AP
            bias_neg_s = persist.tile([P, 1], f32)
            nc.vector.memset(bias_neg_s[:], -S)

            # target indices laid out [partition p, group g] = target[g*P + p]
            tgt_i = persist.tile([P, G], i32)
            nc.sync.dma_start(
                out=tgt_i[:], in_=target[:].rearrange("(g p) -> p g", p=P)
            )

            # flat gather indices within a group: p*C + target  (< 2^24, f32-exact)
            rowoff_i = persist.tile([P, 1], i32)
            nc.gpsimd.iota(rowoff_i[:], pattern=[[0, 1]], base=0,
                           channel_multiplier=n_classes)
            rowoff_f = persist.tile([P, 1], f32)
            nc.vector.tensor_copy(out=rowoff_f[:], in_=rowoff_i[:])
            tgt_f = persist.tile([P, G], f32)
            nc.vector.tensor_copy(out=tgt_f[:], in_=tgt_i[:])
            flat_f = persist.tile([P, G], f32)
            nc.vector.tensor_scalar(
                out=flat_f[:], in0=tgt_f[:], scalar1=rowoff_f[:, 0:1], scalar2=None,
                op0=Alu.add,
            )
            flat_i = persist.tile([P, G], i32)
            nc.vector.tensor_copy(out=flat_i[:], in_=flat_f[:])

            # gather pred[row, target[row]] for each row -> t_raw[p, g]
            pred_flat = pred[:].rearrange("n c -> (n c)")[:, None]
            t_raw = persist.tile([P, G], f32)
            for g in range(G):
                nc.gpsimd.indirect_dma_start(
                    out=t_raw[:, g:g + 1],
                    out_offset=None,
                    in_=pred_flat,
                    in_offset=bass.IndirectOffsetOnAxis(ap=flat_i[:, g:g + 1], axis=0),
                    element_offset=g * P * n_classes,
                )

            # --- hot loop: stream pred, clip-top, exp, accumulate row sums ---
            for g in range(G):
                for k in range(K):
                    x = xin_pool.tile([P, f_chunk], f32)
                    nc.sync.dma_start(
                        out=x[:],
                        in_=pred[g * P:(g + 1) * P, k * f_chunk:(k + 1) * f_chunk],
                    )
                    nc.vector.tensor_scalar_min(out=x[:], in0=x[:], scalar1=1.0)
                    e = edump_pool.tile([P, f_chunk], f32)
                    nc.scalar.activation(
                        out=e[:], in_=x[:], func=Act.Exp, bias=bias_neg_s[:], scale=S,
                        accum_out=rs_partial[:, g * K + k:g * K + k + 1],
                    )

            # --- epilogue (all on [P, G] / [P, 1] tensors) ---
            rs = persist.tile([P, G], f32)
            nc.vector.tensor_reduce(
                out=rs[:], in_=rs_partial[:].rearrange("p (g k) -> p g k", k=K),
                axis=X, op=Alu.add,
            )

            # e_t: the term the hot loop added for the target element
            m_t = persist.tile([P, G], f32)
            nc.vector.tensor_scalar_min(out=m_t[:], in0=t_raw[:], scalar1=1.0)
            e_t = persist.tile([P, G], f32)
            nc.scalar.activation(out=e_t[:], in_=m_t[:], func=Act.Exp,
                                 bias=bias_neg_s[:], scale=S)

            # tgt_cos = clip(t_raw, -1, 1)
            tgt_cos = persist.tile([P, G], f32)
            nc.vector.tensor_scalar(
                out=tgt_cos[:], in0=t_raw[:], scalar1=1.0, scalar2=-1.0,
                op0=Alu.min, op1=Alu.max,
            )
            # sqrt(1 - t^2) = exp(0.5*ln(max(1-t^2, eps)))
            u = persist.tile([P, G], f32)
            nc.vector.tensor_tensor(out=u[:], in0=tgt_cos[:], in1=tgt_cos[:],
                                    op=Alu.mult)
            nc.vector.tensor_scalar(
                out=u[:], in0=u[:], scalar1=-1.0, scalar2=1.0,
                op0=Alu.mult, op1=Alu.add,
            )  # u = 1 - t^2
            nc.vector.tensor_scalar_max(out=u[:], in0=u[:], scalar1=1e-12)
            lnu = persist.tile([P, G], f32)
            nc.scalar.activation(out=lnu[:], in_=u[:], func=Act.Ln)
            sq = persist.tile([P, G], f32)
            nc.scalar.activation(out=sq[:], in_=lnu[:], func=Act.Exp, scale=0.5)

            # tgt_m = where(tgt_cos > THRESHOLD, t*cosM - sinM*sq, tgt_cos - MM)
            cosm_t = persist.tile([P, G], f32)
            nc.vector.tensor_scalar_mul(out=cosm_t[:], in0=tgt_cos[:], scalar1=COS_M)
            tgt_m_raw = persist.tile([P, G], f32)
            nc.vector.scalar_tensor_tensor(
                out=tgt_m_raw[:], in0=sq[:], scalar=-SIN_M, op0=Alu.mult,
                in1=cosm_t[:], op1=Alu.add,
            )
            mask = persist.tile([P, G], mybir.dt.uint8)
            nc.vector.tensor_scalar(
                out=mask[:], in0=tgt_cos[:], scalar1=THRESHOLD, scalar2=None,
                op0=Alu.is_gt,
            )
            alt = persist.tile([P, G], f32)
            nc.vector.tensor_scalar_add(out=alt[:], in0=tgt_cos[:], scalar1=-MM)
            tgt_m = persist.tile([P, G], f32)
            nc.vector.select(out=tgt_m[:], mask=mask[:], on_true=tgt_m_raw[:],
                             on_false=alt[:])

            e_m = persist.tile([P, G], f32)
            nc.scalar.activation(out=e_m[:], in_=tgt_m[:], func=Act.Exp,
                                 bias=bias_neg_s[:], scale=S)

            # s' = rs - e_t + e_m ;  loss = S + ln(s') - S*tgt_m
            nc.vector.tensor_tensor(out=rs[:], in0=rs[:], in1=e_t[:],
                                    op=Alu.subtract)
            nc.vector.tensor_tensor(out=rs[:], in0=rs[:], in1=e_m[:], op=Alu.add)
            ln_s = persist.tile([P, G], f32)
            nc.scalar.activation(out=ln_s[:], in_=rs[:], func=Act.Ln)
            loss = persist.tile([P, G], f32)
            nc.vector.scalar_tensor_tensor(
                out=loss[:], in0=tgt_m[:], scalar=-S, op0=Alu.mult,
                in1=ln_s[:], op1=Alu.add,
            )
            nc.vector.tensor_scalar_add(out=loss[:], in0=loss[:], scalar1=S)

            # per-core scalar: sum over all rows = ones^T @ rowsums
            loss_rowsum = persist.tile([P, 1], f32)
            nc.vector.tensor_reduce(out=loss_rowsum[:], in_=loss[:], axis=X,
                                    op=Alu.add)
            ones = persist.tile([P, 1], f32)
            nc.vector.memset(ones[:], 1.0)
            ps = psum_pool.tile([1, 1], f32)
            nc.tensor.matmul(out=ps[:], lhsT=loss_rowsum[:], rhs=ones[:],
                             start=True, stop=True)
            out_s = persist.tile([1, 1], f32)
            nc.vector.tensor_copy(out=out_s[:], in_=ps[:])
            nc.sync.dma_start(out=out[:, :], in_=out_s[:])

    nc.finalize()
    return nc


_CACHE = {}


def _get_nc():
    if "nc" not in _CACHE:
        _CACHE["nc"] = build_nc()
    return _CACHE["nc"]


def kernel(pred, target):
    from concourse.bass_utils import run_bass_kernel_spmd

    pred = np.ascontiguousarray(np.asarray(pred, dtype=np.float32))
    tgt = np.ascontiguousarray(np.asarray(target).astype(np.int32))
    assert pred.shape == (N, C) and tgt.shape == (N,)

    in_maps = [
        {
            "pred": pred[c * N_SHARD:(c + 1) * N_SHARD],
            "target": tgt[c * N_SHARD:(c + 1) * N_SHARD],
        }
        for c in range(N_CORES)
    ]
    nc = _get_nc()
    res = run_bass_kernel_spmd(nc, in_maps, core_ids=list(range(N_CORES)))
    partials = [np.asarray(r["out"], dtype=np.float64).reshape(-1)[0]
                for r in res.results]
    return np.float32(np.sum(partials) / N)
